# revision 12
# baseline (speedup 1.0000x reference)
"""OAdder2d_Q (oconv, 16-bit dorefa quant) as an 8-core Trainium2 Bass kernel.

Math: with ideal disks the op is a 3x3/pad1 conv with effective kernel
k = w_q * sin(phi_c), sin(phi_c) = s_c = +-1 per input channel.

fp8 formulation (validated to 8e-3 max rel err vs 2e-2 budget):
  sign-fold the input:      xhat = s_c * (x - 0.5)        in [-0.5, 0.5]
  center the weights:       r    = w_q - 0.5              (std ~0.1)
  y(o,p) = conv(xhat, r)(o,p) + z(p) + bias(o,p)
where conv(xhat, r) runs on device in fp8e4m3 with DoubleRow perf mode
(2 conv taps per matmul, 256-deep contraction, 2x PE throughput), and the
two low-rank correction terms are exact host-side elementwise work:
  z(p)      = 0.5 * boxsum3x3(sum_c s_c * x(c,p))         (O-independent)
  bias(o,p) = 0.5 * sum_d m(p+d) * sum_c s_c w_q(o,c,d)   (input-independent)
Device output is fp16 (ulp 0.016 at the +-26 output scale, ~3e-4 rel)
to halve the output DMA traffic.

Sharding: data-parallel over batch, 32 images -> 4 per core, weights
replicated.
"""

import sys

if "/opt/trn_rl_repo" not in sys.path:
    sys.path.insert(0, "/opt/trn_rl_repo")

import ml_dtypes
import numpy as np

import concourse.bacc as bacc
import concourse.mybir as mybir
from concourse.ap import AP
from concourse.tile import TileContext
from concourse.bass_utils import run_bass_kernel_spmd

N_CORES = 8
B, C, O, K, H, W = 32, 128, 256, 3, 56, 56
PB = B // N_CORES              # images per core
HP, WP = H + 2, W + 2          # padded spatial
RB = 8                         # output rows per psum tile
NRB = H // RB                  # row blocks per image
QN = 65535.0                   # 2^16 - 1
WARMUP = 64                    # PE warmup matmuls (ends ~ DVFS ramp done)

# DoubleRow tap pairs (ki*3+kj indices): 4 pairs + single tap 8.
# Second slot offset within the padded tile = delta row * WP + delta col.
PAIRS = [((0, 0), (0, 1)), ((0, 2), (1, 0)), ((1, 1), (1, 2)), ((2, 0), (2, 1))]
SINGLE = (2, 2)

f32 = mybir.dt.float32
f16 = mybir.dt.float16
f8 = mybir.dt.float8e4
E4M3 = ml_dtypes.float8_e4m3fn

_CACHE = {}


def _build_nc():
    nc = bacc.Bacc("TRN2", target_bir_lowering=False, debug=False,
                   num_devices=N_CORES)
    x = nc.dram_tensor("x", (PB, C, HP, WP), f8, kind="ExternalInput")
    w = nc.dram_tensor("w", (C, 5, 2, O), f8, kind="ExternalInput")
    y = nc.dram_tensor("y", (PB, O, H, W), f16, kind="ExternalOutput")

    # img0 input row chunks, halo-aligned so chunk k unlocks row-block k:
    # rb k's matmuls read padded rows [8k, 8k+10)
    CH0 = [(0, 10)] + [(8 * k + 2, 8 * k + 10) for k in range(1, NRB)]

    def pair_rhs(xp, rb, pair):
        (ki0, kj0), (ki1, kj1) = pair
        base = xp[:, rb * RB + ki0: rb * RB + ki0 + RB, kj0: kj0 + W]
        delta = (ki1 - ki0) * WP + (kj1 - kj0)
        ap = [list(base.ap[0]), [delta, 2], list(base.ap[1]), list(base.ap[2])]
        return AP(tensor=base.tensor, offset=base.offset, ap=ap)

    def y_dst(img, rb0, nrb):
        """DRAM dst AP [128, 2, nrb*RB, W]: partition = o % 128, dim1 = o half."""
        base = y[img, 0:128, rb0 * RB:rb0 * RB + nrb * RB, :]
        ap = [list(base.ap[0]), [128 * H * W, 2],
              list(base.ap[1]), list(base.ap[2])]
        return AP(tensor=base.tensor, offset=base.offset, ap=ap)

    with TileContext(nc) as tc:
        with tc.tile_pool(name="wp", bufs=1) as wp, \
             tc.tile_pool(name="xpp", bufs=2) as xpp, \
             tc.tile_pool(name="pp", bufs=7, space="PSUM") as pp, \
             tc.tile_pool(name="wup", bufs=1, space="PSUM") as wup, \
             tc.tile_pool(name="op", bufs=2) as outp:
            # weights first on both HWDGE queues (halves), then img0 chunks
            # on sync -- these gate the first real matmul.
            wt = wp.tile([C, 5, 2, O], f8)
            nc.sync.dma_start(out=wt[:, :2], in_=w[:, 0:2, :, :])
            nc.scalar.dma_start(out=wt[:, 2:], in_=w[:, 2:5, :, :])
            # PE warm-up: dummy matmuls with no data deps so the HAM clock
            # gate is ramped by the time real matmuls start.
            # sized to end right at data-ready (~8.2us) with >=3us of
            # continuous PE activity so the DVFS ramp hits full clock
            wu_in = wp.tile([C, 64], f16)
            nc.vector.memset(wu_in, 0.0)
            wu_ps = wup.tile([32, 64], f32)
            for _ in range(WARMUP):
                nc.tensor.matmul(wu_ps, wu_in[:, :32], wu_in[:, :64],
                                 start=True, stop=True)
            for img in range(PB):
                xp = xpp.tile([C, HP, WP], f8)
                if img == 0:
                    for (r0, r1) in CH0:
                        nc.sync.dma_start(out=xp[:, r0:r1, :],
                                          in_=x[0, :, r0:r1, :])
                else:
                    nc.sync.dma_start(out=xp, in_=x[img, :, :, :])
                yt = outp.tile([128, 2, H, W], f16)
                for rb in range(NRB):
                    for oh in range(O // 128):
                        ps = pp.tile([128, RB, W], f32)
                        for p, pair in enumerate(PAIRS):
                            nc.tensor.matmul(
                                ps, wt[:, p, :, oh * 128:(oh + 1) * 128],
                                pair_rhs(xp, rb, pair),
                                start=(p == 0), stop=False,
                                perf_mode=mybir.MatmulPerfMode.DoubleRow)
                        ki, kj = SINGLE
                        nc.tensor.matmul(
                            ps, wt[:, 4, 0, oh * 128:(oh + 1) * 128],
                            xp[:, rb * RB + ki: rb * RB + ki + RB, kj: kj + W],
                            start=False, stop=True)
                        dst = yt[:, oh, rb * RB:(rb + 1) * RB, :]
                        if img == PB - 1 and rb == NRB - 1:
                            # tail latency: drain the last PSUM tiles with
                            # both engines in parallel
                            nc.vector.tensor_copy(out=dst[:, :RB // 2, :],
                                                  in_=ps[:, :RB // 2, :])
                            nc.scalar.copy(out=dst[:, RB // 2:, :],
                                           in_=ps[:, RB // 2:, :])
                        elif oh % 2 == 0:
                            nc.vector.tensor_copy(out=dst, in_=ps)
                        else:
                            nc.scalar.copy(out=dst, in_=ps)
                    if img == PB - 1:
                        # last image: store per row-block so the final
                        # transfer is small; alternate queues, and split the
                        # very last store across both queues
                        if rb == NRB - 1:
                            nc.sync.dma_start(
                                out=y_dst(img, rb, 1)[:, 0],
                                in_=yt[:, 0, rb * RB:(rb + 1) * RB, :])
                            nc.scalar.dma_start(
                                out=y_dst(img, rb, 1)[:, 1],
                                in_=yt[:, 1, rb * RB:(rb + 1) * RB, :])
                        else:
                            eng = nc.scalar if rb % 2 == 0 else nc.sync
                            eng.dma_start(out=y_dst(img, rb, 1),
                                          in_=yt[:, :, rb * RB:(rb + 1) * RB, :])
                # one store for the whole image (both o halves)
                if img < PB - 1:
                    nc.scalar.dma_start(out=y_dst(img, 0, NRB), in_=yt)
    nc.compile()
    return nc


def _prep_weights(weight, phases, disks):
    """dorefa weight quantize + fold phases/disks; build fp8 residual pack,
    the per-O edge bias map, and generic-disk leftovers."""
    t = np.tanh(weight.astype(np.float32))
    t = t / (2.0 * np.max(np.abs(t))) + 0.5
    wq = (np.round(t * QN) / np.float32(QN)).astype(np.float32)
    s = np.sin(phases.astype(np.float32))[0, 0]        # (C,K,K)
    d0 = disks[0, 0, ..., 0].astype(np.float32)
    d1 = disks[0, 0, ..., 1].astype(np.float32)
    k_mul = wq * (s * (d0 + d1) * 0.5)[None]           # (O,C,K,K) true kernel
    sgn = np.sign(s[:, 0, 0]).astype(np.float32)       # (C,) +-1
    # device weights: r(o,c,d) = s_c*k(o,c,d) - 0.5  (== w_q - 0.5 for
    # ideal disks)
    r = sgn[None, :, None, None] * k_mul - 0.5
    r8 = r.astype(E4M3)
    wpk = np.zeros((C, 5, 2, O), E4M3)
    for p, ((ki0, kj0), (ki1, kj1)) in enumerate(PAIRS):
        wpk[:, p, 0, :] = r8[:, :, ki0, kj0].T
        wpk[:, p, 1, :] = r8[:, :, ki1, kj1].T
    wpk[:, 4, 0, :] = r8[:, :, SINGLE[0], SINGLE[1]].T
    # bias(o,p) = 0.5 * sum_d m(p+d) * A_o(d),  A_o(d) = sum_c k(o,c,d)
    A = k_mul.sum(axis=1)                              # (O,K,K)
    mp = np.zeros((H + 2, W + 2), np.float32)
    mp[1:-1, 1:-1] = 1.0
    bias = np.zeros((O, H, W), np.float32)
    for ki in range(K):
        for kj in range(K):
            bias += A[:, ki, kj][:, None, None] * mp[ki:ki + H, kj:kj + W][None]
    bias *= 0.5
    coef = (d0 - d1) * 0.25                            # zero for ideal disks
    return wpk, bias, sgn, wq, coef


def _prep_x(x, sgn):
    """Host input massage: xhat8 = e4m3(s_c*(x-0.5)) pre-padded, plus the
    exact sign-channel map z = 0.5*boxsum3x3(sum_c s_c x)."""
    xhat = sgn[None, :, None, None] * (x.astype(np.float32) - 0.5)
    xp8 = np.zeros((B, C, HP, WP), E4M3)
    xp8[:, :, 1:-1, 1:-1] = xhat.astype(E4M3)
    u = np.einsum("bchw,c->bhw", x, sgn, optimize=True)
    up = np.zeros((B, HP, WP), np.float32)
    up[:, 1:-1, 1:-1] = u
    z = np.zeros((B, H, W), np.float32)
    for ki in range(K):
        for kj in range(K):
            z += up[:, ki:ki + H, kj:kj + W]
    z *= 0.5
    return xp8, z


def _square_terms(x, wq, coef):
    """Generic-disk correction (zero for ideal disks): conv(x_q^2, coef)
    broadcast over O, plus per-O constant sum(w_q^2 * coef)."""
    xq = np.round(np.clip(x, 0.0, 1.0) * QN) / np.float32(QN)
    x2 = (xq * xq).astype(np.float32)
    bsz = x.shape[0]
    x2p = np.zeros((bsz, C, H + 2, W + 2), np.float32)
    x2p[:, :, 1:H + 1, 1:W + 1] = x2
    y_sq = np.zeros((bsz, H, W), np.float32)
    for ki in range(K):
        for kj in range(K):
            y_sq += np.einsum("bchw,c->bhw",
                              x2p[:, :, ki:ki + H, kj:kj + W],
                              coef[:, ki, kj], optimize=True)
    w_term = np.einsum("ockk,ckk->o", wq * wq, coef)
    return y_sq[:, None] + w_term[None, :, None, None]


def kernel(x, weight, phases, disks):
    x = np.asarray(x)
    wpk, bias, sgn, wq, coef = _prep_weights(
        np.asarray(weight), np.asarray(phases), np.asarray(disks))
    xp8, z = _prep_x(x, sgn)
    if "nc" not in _CACHE:
        _CACHE["nc"] = _build_nc()
    nc = _CACHE["nc"]
    in_maps = [{"x": np.ascontiguousarray(xp8[c * PB:(c + 1) * PB]),
                "w": wpk} for c in range(N_CORES)]
    res = run_bass_kernel_spmd(nc, in_maps, list(range(N_CORES)))
    y16 = np.concatenate([res.results[c]["y"] for c in range(N_CORES)], axis=0)
    y = y16.astype(np.float32)
    y += z[:, None]
    y += bias[None]
    if np.any(coef != 0.0):
        y = y + _square_terms(x, wq, coef)
    return y.astype(np.float32)


# revision 13
# speedup vs baseline: 1.0040x; 1.0040x over previous
"""OAdder2d_Q (oconv, 16-bit dorefa quant) as an 8-core Trainium2 Bass kernel.

Math: with ideal disks the op is a 3x3/pad1 conv with effective kernel
k = w_q * sin(phi_c), sin(phi_c) = s_c = +-1 per input channel.

fp8 formulation (validated to 8e-3 max rel err vs 2e-2 budget):
  sign-fold the input:      xhat = s_c * (x - 0.5)        in [-0.5, 0.5]
  center the weights:       r    = w_q - 0.5              (std ~0.1)
  y(o,p) = conv(xhat, r)(o,p) + z(p) + bias(o,p)
where conv(xhat, r) runs on device in fp8e4m3 with DoubleRow perf mode
(2 conv taps per matmul, 256-deep contraction, 2x PE throughput), and the
two low-rank correction terms are exact host-side elementwise work:
  z(p)      = 0.5 * boxsum3x3(sum_c s_c * x(c,p))         (O-independent)
  bias(o,p) = 0.5 * sum_d m(p+d) * sum_c s_c w_q(o,c,d)   (input-independent)
Device output is fp16 (ulp 0.016 at the +-26 output scale, ~3e-4 rel)
to halve the output DMA traffic.

Sharding: data-parallel over batch, 32 images -> 4 per core, weights
replicated.
"""

import sys

if "/opt/trn_rl_repo" not in sys.path:
    sys.path.insert(0, "/opt/trn_rl_repo")

import ml_dtypes
import numpy as np

import concourse.bacc as bacc
import concourse.mybir as mybir
from concourse.ap import AP
from concourse.tile import TileContext
from concourse.bass_utils import run_bass_kernel_spmd

N_CORES = 8
B, C, O, K, H, W = 32, 128, 256, 3, 56, 56
PB = B // N_CORES              # images per core
HP, WP = H + 2, W + 2          # padded spatial
RB = 8                         # output rows per psum tile
NRB = H // RB                  # row blocks per image
QN = 65535.0                   # 2^16 - 1
WARMUP = 84                    # PE warmup matmuls (ends ~ DVFS ramp done)

# DoubleRow tap pairs (ki*3+kj indices): 4 pairs + single tap 8.
# Second slot offset within the padded tile = delta row * WP + delta col.
PAIRS = [((0, 0), (0, 1)), ((0, 2), (1, 0)), ((1, 1), (1, 2)), ((2, 0), (2, 1))]
SINGLE = (2, 2)

f32 = mybir.dt.float32
f16 = mybir.dt.float16
f8 = mybir.dt.float8e4
E4M3 = ml_dtypes.float8_e4m3fn

_CACHE = {}


def _build_nc():
    nc = bacc.Bacc("TRN2", target_bir_lowering=False, debug=False,
                   num_devices=N_CORES)
    x = nc.dram_tensor("x", (PB, C, HP, WP), f8, kind="ExternalInput")
    w = nc.dram_tensor("w", (C, 5, 2, O), f8, kind="ExternalInput")
    y = nc.dram_tensor("y", (PB, O, H, W), f16, kind="ExternalOutput")

    # img0 input row chunks, halo-aligned so chunk k unlocks row-block k:
    # rb k's matmuls read padded rows [8k, 8k+10)
    CH0 = [(0, 10)] + [(8 * k + 2, 8 * k + 10) for k in range(1, NRB)]

    def pair_rhs(xp, rb, pair):
        (ki0, kj0), (ki1, kj1) = pair
        base = xp[:, rb * RB + ki0: rb * RB + ki0 + RB, kj0: kj0 + W]
        delta = (ki1 - ki0) * WP + (kj1 - kj0)
        ap = [list(base.ap[0]), [delta, 2], list(base.ap[1]), list(base.ap[2])]
        return AP(tensor=base.tensor, offset=base.offset, ap=ap)

    def y_dst(img, rb0, nrb):
        """DRAM dst AP [128, 2, nrb*RB, W]: partition = o % 128, dim1 = o half."""
        base = y[img, 0:128, rb0 * RB:rb0 * RB + nrb * RB, :]
        ap = [list(base.ap[0]), [128 * H * W, 2],
              list(base.ap[1]), list(base.ap[2])]
        return AP(tensor=base.tensor, offset=base.offset, ap=ap)

    with TileContext(nc) as tc:
        with tc.tile_pool(name="wp", bufs=1) as wp, \
             tc.tile_pool(name="xpp", bufs=2) as xpp, \
             tc.tile_pool(name="pp", bufs=7, space="PSUM") as pp, \
             tc.tile_pool(name="wup", bufs=1, space="PSUM") as wup, \
             tc.tile_pool(name="op", bufs=2) as outp:
            # weights first on both HWDGE queues (halves), then img0 chunks
            # on sync -- these gate the first real matmul.
            wt = wp.tile([C, 5, 2, O], f8)
            nc.sync.dma_start(out=wt[:, :2], in_=w[:, 0:2, :, :])
            nc.scalar.dma_start(out=wt[:, 2:], in_=w[:, 2:5, :, :])
            # PE warm-up: dummy matmuls with no data deps so the HAM clock
            # gate is ramped by the time real matmuls start.
            # sized to end right at data-ready (~8.2us) with >=3us of
            # continuous PE activity so the DVFS ramp hits full clock
            wu_in = wp.tile([C, 64], f16)
            nc.vector.memset(wu_in, 0.0)
            wu_ps = wup.tile([32, 64], f32)
            for _ in range(WARMUP):
                nc.tensor.matmul(wu_ps, wu_in[:, :32], wu_in[:, :64],
                                 start=True, stop=True)
            for img in range(PB):
                xp = xpp.tile([C, HP, WP], f8)
                if img == 0:
                    for (r0, r1) in CH0:
                        nc.sync.dma_start(out=xp[:, r0:r1, :],
                                          in_=x[0, :, r0:r1, :])
                else:
                    nc.sync.dma_start(out=xp, in_=x[img, :, :, :])
                yt = outp.tile([128, 2, H, W], f16)
                for rb in range(NRB):
                    for oh in range(O // 128):
                        ps = pp.tile([128, RB, W], f32)
                        for p, pair in enumerate(PAIRS):
                            nc.tensor.matmul(
                                ps, wt[:, p, :, oh * 128:(oh + 1) * 128],
                                pair_rhs(xp, rb, pair),
                                start=(p == 0), stop=False,
                                perf_mode=mybir.MatmulPerfMode.DoubleRow)
                        ki, kj = SINGLE
                        nc.tensor.matmul(
                            ps, wt[:, 4, 0, oh * 128:(oh + 1) * 128],
                            xp[:, rb * RB + ki: rb * RB + ki + RB, kj: kj + W],
                            start=False, stop=True)
                        dst = yt[:, oh, rb * RB:(rb + 1) * RB, :]
                        if img == PB - 1 and rb == NRB - 1:
                            # tail latency: drain the last PSUM tiles with
                            # both engines in parallel
                            nc.vector.tensor_copy(out=dst[:, :RB // 2, :],
                                                  in_=ps[:, :RB // 2, :])
                            nc.scalar.copy(out=dst[:, RB // 2:, :],
                                           in_=ps[:, RB // 2:, :])
                        elif oh % 2 == 0:
                            nc.vector.tensor_copy(out=dst, in_=ps)
                        else:
                            nc.scalar.copy(out=dst, in_=ps)
                    if img == PB - 1:
                        # last image: store per row-block so the final
                        # transfer is small; alternate queues, and split the
                        # very last store across both queues
                        if rb == NRB - 1:
                            nc.sync.dma_start(
                                out=y_dst(img, rb, 1)[:, 0],
                                in_=yt[:, 0, rb * RB:(rb + 1) * RB, :])
                            nc.scalar.dma_start(
                                out=y_dst(img, rb, 1)[:, 1],
                                in_=yt[:, 1, rb * RB:(rb + 1) * RB, :])
                        else:
                            eng = nc.scalar if rb % 2 == 0 else nc.sync
                            eng.dma_start(out=y_dst(img, rb, 1),
                                          in_=yt[:, :, rb * RB:(rb + 1) * RB, :])
                # one store for the whole image (both o halves)
                if img < PB - 1:
                    nc.scalar.dma_start(out=y_dst(img, 0, NRB), in_=yt)
    nc.compile()
    return nc


def _prep_weights(weight, phases, disks):
    """dorefa weight quantize + fold phases/disks; build fp8 residual pack,
    the per-O edge bias map, and generic-disk leftovers."""
    t = np.tanh(weight.astype(np.float32))
    t = t / (2.0 * np.max(np.abs(t))) + 0.5
    wq = (np.round(t * QN) / np.float32(QN)).astype(np.float32)
    s = np.sin(phases.astype(np.float32))[0, 0]        # (C,K,K)
    d0 = disks[0, 0, ..., 0].astype(np.float32)
    d1 = disks[0, 0, ..., 1].astype(np.float32)
    k_mul = wq * (s * (d0 + d1) * 0.5)[None]           # (O,C,K,K) true kernel
    sgn = np.sign(s[:, 0, 0]).astype(np.float32)       # (C,) +-1
    # device weights: r(o,c,d) = s_c*k(o,c,d) - 0.5  (== w_q - 0.5 for
    # ideal disks)
    r = sgn[None, :, None, None] * k_mul - 0.5
    r8 = r.astype(E4M3)
    wpk = np.zeros((C, 5, 2, O), E4M3)
    for p, ((ki0, kj0), (ki1, kj1)) in enumerate(PAIRS):
        wpk[:, p, 0, :] = r8[:, :, ki0, kj0].T
        wpk[:, p, 1, :] = r8[:, :, ki1, kj1].T
    wpk[:, 4, 0, :] = r8[:, :, SINGLE[0], SINGLE[1]].T
    # bias(o,p) = 0.5 * sum_d m(p+d) * A_o(d),  A_o(d) = sum_c k(o,c,d)
    A = k_mul.sum(axis=1)                              # (O,K,K)
    mp = np.zeros((H + 2, W + 2), np.float32)
    mp[1:-1, 1:-1] = 1.0
    bias = np.zeros((O, H, W), np.float32)
    for ki in range(K):
        for kj in range(K):
            bias += A[:, ki, kj][:, None, None] * mp[ki:ki + H, kj:kj + W][None]
    bias *= 0.5
    coef = (d0 - d1) * 0.25                            # zero for ideal disks
    return wpk, bias, sgn, wq, coef


def _prep_x(x, sgn):
    """Host input massage: xhat8 = e4m3(s_c*(x-0.5)) pre-padded, plus the
    exact sign-channel map z = 0.5*boxsum3x3(sum_c s_c x)."""
    xhat = sgn[None, :, None, None] * (x.astype(np.float32) - 0.5)
    xp8 = np.zeros((B, C, HP, WP), E4M3)
    xp8[:, :, 1:-1, 1:-1] = xhat.astype(E4M3)
    u = np.einsum("bchw,c->bhw", x, sgn, optimize=True)
    up = np.zeros((B, HP, WP), np.float32)
    up[:, 1:-1, 1:-1] = u
    z = np.zeros((B, H, W), np.float32)
    for ki in range(K):
        for kj in range(K):
            z += up[:, ki:ki + H, kj:kj + W]
    z *= 0.5
    return xp8, z


def _square_terms(x, wq, coef):
    """Generic-disk correction (zero for ideal disks): conv(x_q^2, coef)
    broadcast over O, plus per-O constant sum(w_q^2 * coef)."""
    xq = np.round(np.clip(x, 0.0, 1.0) * QN) / np.float32(QN)
    x2 = (xq * xq).astype(np.float32)
    bsz = x.shape[0]
    x2p = np.zeros((bsz, C, H + 2, W + 2), np.float32)
    x2p[:, :, 1:H + 1, 1:W + 1] = x2
    y_sq = np.zeros((bsz, H, W), np.float32)
    for ki in range(K):
        for kj in range(K):
            y_sq += np.einsum("bchw,c->bhw",
                              x2p[:, :, ki:ki + H, kj:kj + W],
                              coef[:, ki, kj], optimize=True)
    w_term = np.einsum("ockk,ckk->o", wq * wq, coef)
    return y_sq[:, None] + w_term[None, :, None, None]


def kernel(x, weight, phases, disks):
    x = np.asarray(x)
    wpk, bias, sgn, wq, coef = _prep_weights(
        np.asarray(weight), np.asarray(phases), np.asarray(disks))
    xp8, z = _prep_x(x, sgn)
    if "nc" not in _CACHE:
        _CACHE["nc"] = _build_nc()
    nc = _CACHE["nc"]
    in_maps = [{"x": np.ascontiguousarray(xp8[c * PB:(c + 1) * PB]),
                "w": wpk} for c in range(N_CORES)]
    res = run_bass_kernel_spmd(nc, in_maps, list(range(N_CORES)))
    y16 = np.concatenate([res.results[c]["y"] for c in range(N_CORES)], axis=0)
    y = y16.astype(np.float32)
    y += z[:, None]
    y += bias[None]
    if np.any(coef != 0.0):
        y = y + _square_terms(x, wq, coef)
    return y.astype(np.float32)
